# revision 13
# baseline (speedup 1.0000x reference)
"""DDGCRN cell on 8 TRN2 NeuronCores — data-parallel over batch.

Per core: 8 batches = 16 branch-instances (gate O=128 / update O=64), emitted
as a software pipeline so every engine's static instruction stream stays
dense (engines execute their streams in order; serial per-instance chains
would otherwise stall the TensorEngine and re-throttle its HAM clock gate).

Pipeline: step s emits  P6(s-2) op-matmuls+activation | P4(s-1) dB+yT
| P5(s-1) Lx | P2(s) A-matmuls+relu+rowsum | P3(s) rsqrt+x'+dT | P1(s+1)
hypernet+V.  Update(b) is sequenced >=3 slots after gate(b) (needs z).

Math per instance:
  filt = hypernet MLP (transposed-feature layout, bf16)
  V = tanh(emb*time*day*speed*occupy*filt)      (10, 883)
  A = relu(V V^T) (883,883 symmetric) + fused row-sums (ACT accum_out)
  d = rsqrt(rowsum) via fast-inverse-sqrt + 1 Newton step (DVE only; keeps
      ScalarE pinned to the sigmoid/tanh/relu table set — no table reloads)
  Lx^T = xs^T - dB * ((d*xs)^T A)   using A's symmetry; dB built by 7
      per-row-tile outer products from the transposed d (no DMA broadcast)
  out^T: the einsum sum_{e,k,c} wpool[e,k,c,o] emb[n,e] xg_k[c,n] + bias is
      repacked into 128-row contraction chunks: per e one chunk
      [emb_e*xs[2:66]; emb_e*Lx[2:66]] (one DVE mul against a DMA-stacked
      [xs[2:66];Lx[2:66]] tile), plus one 50-row chunk holding the c<2 rows
      (x / Lx leading rows replicated per e via DMA) and the bias rows
      (emb^T direct).  11 accumulating matmuls per split instead of 21.

All matmuls bf16 (PSUM f32); inputs pre-cast/pre-transposed on host (pure
layout/dtype prep). Output written transposed, un-transposed on host.
"""

import sys, os

sys.path.insert(0, "/opt/trn_rl_repo")

import numpy as np
import ml_dtypes
from contextlib import ExitStack

import concourse.bass as bass
import concourse.bacc as bacc
import concourse.mybir as mybir
from concourse import tile
from concourse.alu_op_type import AluOpType
from concourse.bass_utils import run_bass_kernel_spmd

AF = mybir.ActivationFunctionType
F32 = mybir.dt.float32
BF16 = mybir.dt.bfloat16
FP8 = mybir.dt.float8e4
I32 = mybir.dt.int32
BF16_NP = ml_dtypes.bfloat16
FP8_NP = ml_dtypes.float8_e4m3
NPAD = 896

B, N, DIN, DOUT, E, CHEB = 64, 883, 2, 64, 10, 2
C = DIN + DOUT  # 66
NCORES = 8
BL = B // NCORES  # 8 batches per core
NT = (N + 127) // 128  # 7 row tiles
OG, OU = 2 * DOUT, DOUT  # 128, 64
SPLITS = [(0, 512), (512, N - 512)]
RSQRT_MAGIC = 0x5F3759DF

# instance schedule: update(b) >= 3 slots after gate(b)
SEQ = [("g", 0), ("g", 1), ("g", 2), ("u", 0), ("g", 3), ("u", 1), ("g", 4),
       ("u", 2), ("g", 5), ("u", 3), ("g", 6), ("u", 4), ("g", 7), ("u", 5),
       ("u", 6), ("u", 7)]


def _pt(nt):
    return min(128, N - nt * 128)


def _build_body(tc, ctx, nc, P):
    def pool(name, bufs, space="SBUF"):
        return ctx.enter_context(tc.tile_pool(name=name, bufs=bufs, space=space))

    wp = pool("wp", 1)        # static weights
    dat = pool("dat", 2)      # per-batch DMA loads
    act = pool("act", 2)      # per-instance intermediates
    arp = pool("arp", 14)     # relu(A) tiles: 2 instances x 7 in flight
    xnp = pool("xnp", 29)     # natural xs/cand tiles
    xpp = pool("xpp", 15)     # d*xs tiles
    zcp = pool("zcp", 2)      # einsum contraction chunks
    dnp = pool("dnp", 4)      # rowsum/d helpers
    psp = pool("psp", 4, space="PSUM")  # op + yT accumulators (tag psA)
    psa = pool("psa", 4, space="PSUM")  # A halves / hypernet / transposes

    def ps_pair(p, name, parts, tag):
        return [p.tile([parts, sl], F32, tag=tag, name=f"{name}_{i}")
                for i, (s0, sl) in enumerate(SPLITS)]

    # ---------------- static setup phase A: small tiles the first
    # instance needs immediately (batch-0 input DMAs must not queue
    # behind the bulk weights) ------------------------------------
    ident_f = wp.tile([128, 128], F32, tag="identf", name="ident_f")
    nc.sync.dma_start(ident_f[:, :], P["ident"][:, :])
    ident_b = wp.tile([128, 128], BF16, tag="identb", name="ident_b")
    nc.vector.tensor_copy(ident_b[:, :], ident_f[:, :])
    ones66 = wp.tile([1, C], BF16, tag="ones66", name="ones66")
    nc.vector.memset(ones66[:, :], 1.0)

    def load_bf(pname, shape, tag):
        t = wp.tile(list(shape), BF16, tag=tag, name=pname + "_t")
        nc.sync.dma_start(t[:, :], P[pname][:, :])
        return t

    embT = load_bf("embT", (E, N), "embT")
    fc = {}
    for br in ("g", "u"):
        fc[("w1", br)] = load_bf(f"fc1w_{br}", (C, 16), f"fc1w{br}")
        fc[("w2", br)] = load_bf(f"fc2w_{br}", (16, 2), f"fc2w{br}")
        fc[("w3", br)] = load_bf(f"fc3w_{br}", (2, E), f"fc3w{br}")
        for nm, shape in (("b1", (16, 1)), ("b2", (2, 1)), ("b3", (E, 1))):
            t = wp.tile(list(shape), F32, tag=f"fc{nm}{br}", name=f"fc{nm}{br}")
            nc.sync.dma_start(t[:, :], P[f"fc{nm}_{br}"][:, :])
            fc[(nm, br)] = t

    def statics_phase_b():
        """Bulk weights: first needed by P6(SEQ[0]) ~2 slots in."""
        nonlocal embB128, embSS, wzc, wzs
        embB128 = []
        for e in range(E):
            t = wp.tile([128, N], BF16, tag=f"embB{e}", name=f"embB{e}")
            nc.sync.dma_start(t[:, :], P["embB128"][e * 128 : (e + 1) * 128, :])
            embB128.append(t)
        embSS = wp.tile([52, N], BF16, tag="embSS", name="embSS")
        nc.sync.dma_start(embSS[:, :], P["embSS"][:, :])
        wzc = {}
        wzs = {}
        for br, On in (("g", OG), ("u", OU)):
            tiles = []
            for p in range(E // 2):
                t = wp.tile([128, 2 * On], FP8, tag=f"wz2{br}{p}", name=f"wz2{br}{p}")
                nc.sync.dma_start(t[:, :], P[f"wz2_{br}"][p * 128 : (p + 1) * 128, :])
                tiles.append(t)
            wzc[br] = tiles
            t = wp.tile([52, On], BF16, tag=f"wzs{br}", name=f"wzs{br}")
            nc.sync.dma_start(t[:, :], P[f"wzs_{br}"][:, :])
            wzs[br] = t

    embB128 = embSS = wzc = wzs = None
    DR = mybir.MatmulPerfMode.DoubleRow

    # ---------------- per-instance state ----------------
    ST = {}   # (br,b) -> dict of tiles
    BAT = {}  # b -> dict of per-batch tiles

    def batch_load(b):
        """DMA this batch's inputs; build Mb, xgsx, and the gate XGS base."""
        d = {}
        xs_nat = []
        for nt in range(NT):
            p = _pt(nt)
            t = xnp.tile([128, C], BF16, tag="xsn", name=f"xsn{b}{nt}")
            nc.sync.dma_start(t[:p, :], P["xs_nat"][b, nt * 128 : nt * 128 + p, :])
            xs_nat.append(t)
        d["xs_nat"] = xs_nat
        xsTt = act.tile([C, N], BF16, tag="xsT", name=f"xsT{b}", bufs=4)
        nc.sync.dma_start(xsTt[:, :], P["xsT"][b, :, :])
        d["xsT"] = xsTt
        # gate-branch contraction stack: rows 0:64 = xs[2:66] straight from HBM
        xgs_g = act.tile([128, N], BF16, tag="xgsg", name=f"xgsg{b}", bufs=3)
        nc.sync.dma_start(xgs_g[0:64, :], P["xsT"][b, 2:C, :])
        d["xgs_g"] = xgs_g
        stT = dat.tile([DOUT, N], BF16, tag="stT", name=f"stT{b}", bufs=4)
        nc.sync.dma_start(stT[:, :], P["stateT"][b, :, :])
        d["stT"] = stT
        # x rows replicated per e for the small contraction chunk (both branches)
        xgsx = act.tile([20, N], BF16, tag="xgsx", name=f"xgsx{b}", bufs=4)
        nc.gpsimd.dma_start(xgsx[0:2, :], xsTt[0:DIN, :])
        nc.gpsimd.dma_start(xgsx[2:4, :], xgsx[0:2, :])
        nc.gpsimd.dma_start(xgsx[4:8, :], xgsx[0:4, :])
        nc.gpsimd.dma_start(xgsx[8:16, :], xgsx[0:8, :])
        nc.gpsimd.dma_start(xgsx[16:20, :], xgsx[0:4, :])
        d["xgsx"] = xgsx
        tdso = []
        for nm in ("tT", "dT", "sT", "oT"):
            t = dat.tile([E, N], BF16, tag=nm, name=f"{nm}{b}", bufs=2)
            nc.sync.dma_start(t[:, :], P[nm][b, :, :])
            tdso.append(t)
        p1 = act.tile([E, N], BF16, tag="p1", name=f"p1_{b}", bufs=2)
        nc.vector.tensor_mul(p1[:, :], tdso[0][:, :], tdso[1][:, :])
        p2 = act.tile([E, N], BF16, tag="p2", name=f"p2_{b}", bufs=1)
        nc.vector.tensor_mul(p2[:, :], tdso[2][:, :], tdso[3][:, :])
        p3 = act.tile([E, N], BF16, tag="p1", name=f"p3_{b}", bufs=2)
        nc.vector.tensor_mul(p3[:, :], p1[:, :], p2[:, :])
        Mb = act.tile([E, N], BF16, tag="Mb", name=f"Mb{b}", bufs=4)
        nc.vector.tensor_mul(Mb[:, :], p3[:, :], embT[:, :])
        d["Mb"] = Mb
        BAT[b] = d

    def P1(inst):
        """Hypernet + V. For gate instances, also triggers the batch load."""
        br, b = inst
        if br == "g":
            batch_load(b)
            st = ST[inst] = {}
            st["x0"] = BAT[b]["xsT"]
            st["xgs"] = BAT[b]["xgs_g"]
        else:
            st = ST[inst]  # created by glue(gate): has x0=candT, xgs, r_sb, cn
        xg2 = st["x0"]
        h1p = ps_pair(psa, f"h1p{br}{b}", 16, "psB")
        h1 = act.tile([16, N], BF16, tag="h1", name=f"h1{br}{b}")
        for i, (s0, sl) in enumerate(SPLITS):
            nc.tensor.matmul(h1p[i][:16, :sl], fc[("w1", br)][:, :],
                             xg2[:, s0 : s0 + sl], start=True, stop=True)
            nc.scalar.activation(h1[:, s0 : s0 + sl], h1p[i][:16, :sl],
                                 AF.Sigmoid, bias=fc[("b1", br)][:, :])
        h2p = ps_pair(psa, f"h2p{br}{b}", 2, "psB")
        h2 = act.tile([2, N], BF16, tag="h2", name=f"h2{br}{b}")
        for i, (s0, sl) in enumerate(SPLITS):
            nc.tensor.matmul(h2p[i][:2, :sl], fc[("w2", br)][:, :],
                             h1[:, s0 : s0 + sl], start=True, stop=True)
            nc.scalar.activation(h2[:, s0 : s0 + sl], h2p[i][:2, :sl],
                                 AF.Sigmoid, bias=fc[("b2", br)][:, :])
        h3p = ps_pair(psa, f"h3p{br}{b}", E, "psB")
        filt = act.tile([E, N], BF16, tag="filt", name=f"filt{br}{b}")
        for i, (s0, sl) in enumerate(SPLITS):
            nc.tensor.matmul(h3p[i][:E, :sl], fc[("w3", br)][:, :],
                             h2[:, s0 : s0 + sl], start=True, stop=True)
            nc.vector.tensor_scalar_add(filt[:, s0 : s0 + sl], h3p[i][:E, :sl],
                                        fc[("b3", br)][:, :])
        vpre = act.tile([E, N], BF16, tag="vpre", name=f"vpre{br}{b}")
        nc.vector.tensor_mul(vpre[:, :], BAT[b]["Mb"][:, :], filt[:, :])
        V = act.tile([E, N], BF16, tag="V", name=f"V{br}{b}")
        nc.scalar.activation(V[:, :], vpre[:, :], AF.Tanh)
        st["V"] = V
        rs0 = dnp.tile([128, 8], F32, tag="rs0", name=f"rs0{br}{b}")
        rs1 = dnp.tile([128, 8], F32, tag="rs1", name=f"rs1{br}{b}")
        nc.vector.memset(rs0[:, :], 0.5)
        nc.vector.memset(rs1[:, :], 0.5)
        st["rs"] = (rs0, rs1)

    def P2(inst):
        """A = relu(V V^T) + fused row-sums."""
        br, b = inst
        st = ST[inst]
        V, rsh = st["V"], st["rs"]
        ar = []
        for kt in range(NT):
            p = _pt(kt)
            aps = [psa.tile([128, sl], F32, tag="psB", name=f"aps{br}{b}{kt}_{i}")
                   for i, (s0, sl) in enumerate(SPLITS)]
            art = arp.tile([128, N], BF16, tag="ar", name=f"ar{br}{b}{kt}")
            for i, (s0, sl) in enumerate(SPLITS):
                nc.tensor.matmul(aps[i][:p, :sl],
                                 V[:, kt * 128 : kt * 128 + p],
                                 V[:, s0 : s0 + sl], start=True, stop=True)
                nc.scalar.activation(art[:p, s0 : s0 + sl], aps[i][:p, :sl],
                                     AF.Relu, accum_out=rsh[i][:p, kt : kt + 1])
            ar.append(art)
        st["ar"] = ar

    def P3(inst):
        """d = rsqrt(rowsums) on DVE; x' = d*xs; transposed d row (drs)."""
        br, b = inst
        st = ST[inst]
        rs0, rs1 = st["rs"]
        rsall = dnp.tile([128, 8], F32, tag="rsall", name=f"rsall{br}{b}")
        nc.vector.tensor_add(rsall[:, :], rs0[:, :], rs1[:, :])
        tsh = dnp.tile([128, 8], F32, tag="tsh", name=f"tsh{br}{b}")
        nc.vector.tensor_scalar(tsh[:, :].bitcast(I32), rsall[:, :].bitcast(I32),
                                1, None, AluOpType.logical_shift_right)
        tnot = dnp.tile([128, 8], F32, tag="tnot", name=f"tnot{br}{b}")
        nc.vector.tensor_scalar(tnot[:, :].bitcast(I32), tsh[:, :].bitcast(I32),
                                -1, None, AluOpType.bitwise_xor)
        d0 = dnp.tile([128, 8], F32, tag="d0", name=f"d0{br}{b}")
        nc.vector.tensor_scalar(d0[:, :].bitcast(I32), tnot[:, :].bitcast(I32),
                                RSQRT_MAGIC + 1, None, AluOpType.add)
        sq = dnp.tile([128, 8], F32, tag="sq", name=f"sq{br}{b}")
        nc.vector.tensor_mul(sq[:, :], d0[:, :], d0[:, :])
        hx = dnp.tile([128, 8], F32, tag="hx", name=f"hx{br}{b}")
        nc.vector.tensor_mul(hx[:, :], sq[:, :], rsall[:, :])
        cf = dnp.tile([128, 8], F32, tag="cf", name=f"cf{br}{b}")
        nc.vector.tensor_scalar(cf[:, :], hx[:, :], -0.5, 1.5,
                                AluOpType.mult, AluOpType.add)
        dcat = dnp.tile([128, 8], F32, tag="dcat", name=f"dcat{br}{b}")
        nc.vector.tensor_mul(dcat[:, :], d0[:, :], cf[:, :])
        st["dcat"] = dcat
        xnat = BAT[b]["xs_nat"] if br == "g" else st["cn"]
        xp = []
        for kt in range(NT):
            p = _pt(kt)
            xpt = xpp.tile([128, C], BF16, tag="xp", name=f"xp{br}{b}{kt}")
            nc.vector.tensor_scalar_mul(xpt[:p, :], xnat[kt][:p, :],
                                        dcat[:p, kt : kt + 1])
            xp.append(xpt)
        st["xp"] = xp
        # transposed d row, one slot ahead of P4's dB outer products
        tp = psa.tile([128, 128], F32, tag="psB", name=f"dtp{br}{b}")
        nc.tensor.transpose(tp[:8, :128], dcat[:, :], ident_f[:, :])
        drs = act.tile([8, 128], BF16, tag="drs", name=f"drs{br}{b}")
        nc.vector.tensor_copy(drs[:, :], tp[:8, :128])
        st["drs7"] = drs

    def P4(inst):
        """y^T matmuls + dB via per-row-tile outer products from drs."""
        br, b = inst
        st = ST[inst]
        drow = act.tile([1, 896], BF16, tag="drow", name=f"drow{br}{b}")
        nc.sync.dma_start(drow[0:1, :], st["drs7"][0:7, :])
        st["drow"] = drow
        yt = ps_pair(psp, f"yt{br}{b}", C, "psA")
        ar, xp = st["ar"], st["xp"]
        for kt in range(NT):
            p = _pt(kt)
            for i, (s0, sl) in enumerate(SPLITS):
                nc.tensor.matmul(yt[i][:C, :sl], xp[kt][:p, :],
                                 ar[kt][:p, s0 : s0 + sl],
                                 start=(kt == 0), stop=(kt == NT - 1))
        st["yt"] = yt
        drow = st["drow"]
        dbp = [psa.tile([128, sl], F32, tag="psB", name=f"dbp{br}{b}_{i}")
               for i, (s0, sl) in enumerate(SPLITS)]
        dB = act.tile([C, N], BF16, tag="dB", name=f"dB{br}{b}")
        for i, (s0, sl) in enumerate(SPLITS):
            nc.tensor.matmul(dbp[i][:C, :sl], ones66[:, :],
                             drow[0:1, s0 : s0 + sl], start=True, stop=True)
            nc.scalar.copy(dB[:, s0 : s0 + sl], dbp[i][:C, :sl])
        st["dB"] = dB

    def P5(inst):
        """Lx^T = xsT - dB*y^T; stack Lx rows into the contraction tiles."""
        br, b = inst
        st = ST[inst]
        yt, dB = st["yt"], st["dB"]
        yd = act.tile([C, N], BF16, tag="yd", name=f"yd{br}{b}")
        for i, (s0, sl) in enumerate(SPLITS):
            nc.vector.tensor_mul(yd[:, s0 : s0 + sl], yt[i][:C, :sl],
                                 dB[:, s0 : s0 + sl])
        lxT = act.tile([C, N], BF16, tag="lxT", name=f"lxT{br}{b}", bufs=3)
        nc.vector.tensor_sub(lxT[:, :], st["x0"][:, :], yd[:, :])
        st["x1"] = lxT
        nc.gpsimd.dma_start(st["xgs"][64:128, :], lxT[2:C, :])
        xg1 = act.tile([52, N], BF16, tag="xg1", name=f"xg1{br}{b}", bufs=3)
        nc.gpsimd.dma_start(xg1[32:34, :], lxT[0:DIN, :])
        nc.gpsimd.dma_start(xg1[34:36, :], xg1[32:34, :])
        nc.gpsimd.dma_start(xg1[36:40, :], xg1[32:36, :])
        nc.gpsimd.dma_start(xg1[40:48, :], xg1[32:40, :])
        nc.gpsimd.dma_start(xg1[48:52, :], xg1[32:36, :])
        st["xg1"] = xg1

    def P6(inst):
        """Final per-node einsum (128-row repacked contraction) + activation."""
        br, b = inst
        st = ST[inst]
        On = OG if br == "g" else OU
        outf = AF.Sigmoid if br == "g" else AF.Tanh
        xgs = st["xgs"]
        op = ps_pair(psp, f"op{br}{b}", On, "psA")
        for pr in range(E // 2):
            stage = zcp.tile([128, 2 * NPAD], BF16, tag="zst8",
                             name=f"zstg{br}{b}{pr}")
            nc.vector.tensor_mul(stage[:, 0:N], embB128[2 * pr][:, :], xgs[:, :])
            nc.vector.tensor_mul(stage[:, NPAD : NPAD + N],
                                 embB128[2 * pr + 1][:, :], xgs[:, :])
            z2 = zcp.tile([128, 2 * NPAD], FP8, tag="z2f8", name=f"z2{br}{b}{pr}")
            nc.gpsimd.dma_start(z2[:, :], stage[:, :])
            z3 = z2[:, :].rearrange("q (k n) -> q k n", k=2)
            w3 = wzc[br][pr][:, :].rearrange("q (k o) -> q k o", k=2)
            for i, (s0, sl) in enumerate(SPLITS):
                nc.tensor.matmul(op[i][:On, :sl], w3[:, :, 0:On],
                                 z3[:, :, s0 : s0 + sl], start=(pr == 0),
                                 stop=False, perf_mode=DR)
        small = zcp.tile([52, N], BF16, tag="small", name=f"small{br}{b}")
        nc.vector.tensor_mul(small[0:20, :], embSS[0:20, :],
                             BAT[b]["xgsx"][0:20, :])
        nc.gpsimd.dma_start(small[20:32, :], P["embTz"][:, :])
        nc.vector.tensor_mul(small[32:52, :], embSS[32:52, :],
                             st["xg1"][32:52, :])
        for i, (s0, sl) in enumerate(SPLITS):
            nc.tensor.matmul(op[i][:On, :sl], wzs[br][0:52, :On],
                             small[0:52, s0 : s0 + sl], start=False, stop=True)
        zout = act.tile([On, N], BF16, tag=f"zout{br}", name=f"zout{br}{b}")
        for i, (s0, sl) in enumerate(SPLITS):
            nc.scalar.activation(zout[:, s0 : s0 + sl], op[i][:On, :sl], outf)
        st["zout"] = zout

    def glue(inst):
        """After P6: gate -> build update inputs; update -> epilogue + store."""
        br, b = inst
        if br == "g":
            zr = ST[inst]["zout"]  # (128, N): z rows 0:64, r rows 64:128
            stT = BAT[b]["stT"]
            ust = {}
            ST[("u", b)] = ust
            zst = act.tile([DOUT, N], BF16, tag="zst", name=f"zst{b}")
            nc.vector.tensor_mul(zst[:, :], zr[0:DOUT, :], stT[:, :])
            candT = act.tile([C, N], BF16, tag="candT", name=f"candT{b}", bufs=3)
            nc.gpsimd.dma_start(candT[0:DIN, :], BAT[b]["xsT"][0:DIN, :])
            nc.gpsimd.dma_start(candT[DIN:C, :], zst[:, :])
            ust["x0"] = candT
            xgs_u = act.tile([128, N], BF16, tag="xgsu", name=f"xgsu{b}", bufs=3)
            nc.gpsimd.dma_start(xgs_u[0:64, :], zst[:, :])
            ust["xgs"] = xgs_u
            r_sb = act.tile([DOUT, N], BF16, tag="r_sb", name=f"r_sb{b}", bufs=3)
            nc.gpsimd.dma_start(r_sb[:, :], zr[DOUT:OG, :])
            ust["r_sb"] = r_sb
            cn_l = []
            for nt in range(NT):
                p = _pt(nt)
                zps = psa.tile([128, 128], BF16, tag="psB", name=f"znp{b}{nt}")
                nc.tensor.transpose(zps[:p, :DOUT],
                                    zr[0:DOUT, nt * 128 : nt * 128 + p],
                                    ident_b[:DOUT, :DOUT])
                zn = act.tile([128, DOUT], BF16, tag="zn", name=f"zn{b}{nt}",
                              bufs=4)
                nc.vector.tensor_copy(zn[:p, :], zps[:p, :DOUT])
                cn = xnp.tile([128, C], BF16, tag="cn", name=f"cn{b}{nt}", bufs=22)
                nc.gpsimd.tensor_copy(cn[:p, 0:DIN],
                                      BAT[b]["xs_nat"][nt][:p, 0:DIN])
                nc.gpsimd.tensor_mul(cn[:p, DIN:C], zn[:p, :],
                                     BAT[b]["xs_nat"][nt][:p, DIN:C])
                cn_l.append(cn)
            ust["cn"] = cn_l
        else:
            hc = ST[inst]["zout"]  # (64, N)
            stT = BAT[b]["stT"]
            r_sb = ST[inst]["r_sb"]
            t1 = act.tile([OU, N], BF16, tag="t1", name=f"t1_{b}", bufs=2)
            nc.gpsimd.tensor_sub(t1[:, :], stT[:, :], hc[:, :])
            t2 = act.tile([OU, N], BF16, tag="t2", name=f"t2_{b}", bufs=2)
            nc.gpsimd.tensor_mul(t2[:, :], r_sb[:, :], t1[:, :])
            outT = act.tile([OU, N], F32, tag="outT", name=f"outT{b}")
            nc.vector.tensor_add(outT[:, :], t2[:, :], hc[:, :])
            nc.sync.dma_start(P["out"][b, :, :], outT[:, :])

    # ---------------- pipeline driver ----------------
    M = len(SEQ)
    P1(SEQ[0])
    for s in range(M + 2):
        if 0 <= s - 2 < M:
            P6(SEQ[s - 2])
            glue(SEQ[s - 2])
        if 0 <= s - 1 < M:
            P4(SEQ[s - 1])
            P5(SEQ[s - 1])
        if s < M:
            P2(SEQ[s])
            P3(SEQ[s])
        if s + 1 < M:
            P1(SEQ[s + 1])
        if s == 0:
            statics_phase_b()


def build_nc():
    nc = bacc.Bacc()
    P = {}

    def dp(name, shape, dtype=F32, out=False):
        P[name] = nc.declare_dram_parameter(name, list(shape), dtype, isOutput=out)

    dp("xs_nat", (BL, N, C), BF16)
    dp("xsT", (BL, C, N), BF16)
    dp("stateT", (BL, DOUT, N), BF16)
    for nm in ("tT", "dT", "sT", "oT"):
        dp(nm, (BL, E, N), BF16)
    dp("embT", (E, N), BF16)
    dp("embB128", (E * 128, N), BF16)
    dp("embSS", (52, N), BF16)
    dp("embTz", (12, N), BF16)
    dp("wz2_g", (E // 2 * 128, 4 * DOUT), FP8)
    dp("wz2_u", (E // 2 * 128, 2 * DOUT), FP8)
    dp("wzs_g", (52, OG), BF16)
    dp("wzs_u", (52, OU), BF16)
    for br in ("g", "u"):
        dp(f"fc1w_{br}", (C, 16), BF16)
        dp(f"fc2w_{br}", (16, 2), BF16)
        dp(f"fc3w_{br}", (2, E), BF16)
        dp(f"fcb1_{br}", (16, 1))
        dp(f"fcb2_{br}", (2, 1))
        dp(f"fcb3_{br}", (E, 1))
    dp("ident", (128, 128))
    dp("out", (BL, OU, N), F32, out=True)
    with tile.TileContext(nc) as tc:
        with ExitStack() as ctx:
            _build_body(tc, ctx, nc, P)
    nc.finalize()
    return nc


_NC_CACHE = {}


def _get_nc():
    if "nc" not in _NC_CACHE:
        _NC_CACHE["nc"] = build_nc()
    return _NC_CACHE["nc"]


def _make_in_maps(inputs):
    f32 = lambda a: np.ascontiguousarray(a, dtype=np.float32)
    bf = lambda a: np.ascontiguousarray(np.asarray(a, dtype=np.float32).astype(BF16_NP))
    x = f32(inputs["x"])
    state = f32(inputs["state"])
    emb = f32(inputs["node_embeddings"])
    time, day = f32(inputs["time"]), f32(inputs["day"])
    speed, occupy = f32(inputs["speed"]), f32(inputs["occupy"])
    xs = np.concatenate([x, state], axis=-1)
    embT = emb.T  # (E, N)
    embB128 = np.repeat(embT[:, None, :], 128, axis=1).reshape(E * 128, N)
    emb_pairs = np.repeat(embT[:, None, :], 2, axis=1).reshape(2 * E, N)
    embSS = np.concatenate([emb_pairs, np.zeros((12, N), np.float32), emb_pairs],
                           axis=0)
    embTz = np.concatenate([embT, np.zeros((2, N), np.float32)], axis=0)

    def pack_w(wpool, bpool, On):
        # main chunks: per e [wpool[e,0,2:66]; wpool[e,1,2:66]] -> (1280, On)
        wzc = np.concatenate(
            [np.concatenate([wpool[e, 0, DIN:C], wpool[e, 1, DIN:C]], axis=0)
             for e in range(E)], axis=0)
        # small chunk: x rows (k=0,c<2) per e, Lx rows (k=1,c<2) per e, bias
        w_x = wpool[:, 0, 0:DIN, :].reshape(E * DIN, On)
        w_l = wpool[:, 1, 0:DIN, :].reshape(E * DIN, On)
        wzs = np.concatenate([w_x, bpool, np.zeros((2, On), np.float32), w_l],
                             axis=0)
        return wzc, wzs

    wzc_g, wzs_g = pack_w(f32(inputs["gate_wpool"]), f32(inputs["gate_bpool"]), OG)
    wzc_u, wzs_u = pack_w(f32(inputs["update_wpool"]), f32(inputs["update_bpool"]), OU)

    def pair_w(wzc, On):
        return np.concatenate(
            [np.concatenate([wzc[2 * p * 128 : (2 * p + 1) * 128],
                             wzc[(2 * p + 1) * 128 : (2 * p + 2) * 128]], axis=1)
             for p in range(E // 2)], axis=0)

    f8 = lambda a: np.ascontiguousarray(np.asarray(a, dtype=np.float32).astype(FP8_NP))
    wz2_g = pair_w(wzc_g, OG)
    wz2_u = pair_w(wzc_u, OU)

    shared = {
        "embT": bf(embT),
        "embB128": bf(embB128),
        "embSS": bf(embSS),
        "embTz": bf(embTz),
        "wz2_g": f8(wz2_g),
        "wz2_u": f8(wz2_u),
        "wzs_g": bf(wzs_g),
        "wzs_u": bf(wzs_u),
        "ident": np.eye(128, dtype=np.float32),
    }
    for br, pre in (("g", "gate"), ("u", "update")):
        shared[f"fc1w_{br}"] = bf(inputs[f"{pre}_fc1_w"])
        shared[f"fc2w_{br}"] = bf(inputs[f"{pre}_fc2_w"])
        shared[f"fc3w_{br}"] = bf(inputs[f"{pre}_fc3_w"])
        shared[f"fcb1_{br}"] = f32(inputs[f"{pre}_fc1_b"].reshape(16, 1))
        shared[f"fcb2_{br}"] = f32(inputs[f"{pre}_fc2_b"].reshape(2, 1))
        shared[f"fcb3_{br}"] = f32(inputs[f"{pre}_fc3_b"].reshape(E, 1))

    in_maps = []
    for c in range(NCORES):
        sl = slice(c * BL, (c + 1) * BL)
        m = dict(shared)
        m["xs_nat"] = bf(xs[sl])
        m["xsT"] = bf(xs[sl].transpose(0, 2, 1))
        m["stateT"] = bf(state[sl].transpose(0, 2, 1))
        m["tT"] = bf(time[sl].transpose(0, 2, 1))
        m["dT"] = bf(day[sl].transpose(0, 2, 1))
        m["sT"] = bf(speed[sl].transpose(0, 2, 1))
        m["oT"] = bf(occupy[sl].transpose(0, 2, 1))
        in_maps.append(m)
    return in_maps


def _run(inputs, trace=False):
    nc = _get_nc()
    in_maps = _make_in_maps(inputs)
    res = run_bass_kernel_spmd(nc, in_maps, core_ids=list(range(NCORES)), trace=trace)
    out = np.concatenate(
        [np.asarray(res.results[i]["out"]).transpose(0, 2, 1) for i in range(NCORES)],
        axis=0,
    )
    return out.astype(np.float32), res


def kernel(**inputs):
    out, _ = _run(inputs, trace=False)
    return out


# revision 14
# speedup vs baseline: 1.1892x; 1.1892x over previous
"""DDGCRN cell on 8 TRN2 NeuronCores — data-parallel over batch.

Per core: 8 batches = 16 branch-instances (gate O=128 / update O=64), emitted
as a software pipeline so every engine's static instruction stream stays
dense (engines execute their streams in order; serial per-instance chains
would otherwise stall the TensorEngine and re-throttle its HAM clock gate).

Pipeline: step s emits  P6(s-2) op-matmuls+activation | P4(s-1) dB+yT
| P5(s-1) Lx | P2(s) A-matmuls+relu+rowsum | P3(s) rsqrt+x'+dT | P1(s+1)
hypernet+V.  Update(b) is sequenced >=3 slots after gate(b) (needs z).

Math per instance:
  filt = hypernet MLP (transposed-feature layout, bf16)
  V = tanh(emb*time*day*speed*occupy*filt)      (10, 883)
  A = relu(V V^T) (883,883 symmetric) + fused row-sums (ACT accum_out)
  d = rsqrt(rowsum) via fast-inverse-sqrt + 1 Newton step (DVE only; keeps
      ScalarE pinned to the sigmoid/tanh/relu table set — no table reloads)
  Lx^T = xs^T - dB * ((d*xs)^T A)   using A's symmetry; dB built by 7
      per-row-tile outer products from the transposed d (no DMA broadcast)
  out^T: the einsum sum_{e,k,c} wpool[e,k,c,o] emb[n,e] xg_k[c,n] + bias is
      repacked into 128-row contraction chunks: per e one chunk
      [emb_e*xs[2:66]; emb_e*Lx[2:66]] (one DVE mul against a DMA-stacked
      [xs[2:66];Lx[2:66]] tile), plus one 50-row chunk holding the c<2 rows
      (x / Lx leading rows replicated per e via DMA) and the bias rows
      (emb^T direct).  11 accumulating matmuls per split instead of 21.

All matmuls bf16 (PSUM f32); inputs pre-cast/pre-transposed on host (pure
layout/dtype prep). Output written transposed, un-transposed on host.
"""

import sys, os

sys.path.insert(0, "/opt/trn_rl_repo")

import numpy as np
import ml_dtypes
from contextlib import ExitStack

import concourse.bass as bass
import concourse.bacc as bacc
import concourse.mybir as mybir
from concourse import tile
from concourse.alu_op_type import AluOpType
from concourse.bass_utils import run_bass_kernel_spmd

AF = mybir.ActivationFunctionType
F32 = mybir.dt.float32
BF16 = mybir.dt.bfloat16
FP8 = mybir.dt.float8e4
I32 = mybir.dt.int32
BF16_NP = ml_dtypes.bfloat16
FP8_NP = ml_dtypes.float8_e4m3
NPAD = 896

B, N, DIN, DOUT, E, CHEB = 64, 883, 2, 64, 10, 2
C = DIN + DOUT  # 66
NCORES = 8
BL = B // NCORES  # 8 batches per core
NT = (N + 127) // 128  # 7 row tiles
OG, OU = 2 * DOUT, DOUT  # 128, 64
SPLITS = [(0, 512), (512, N - 512)]
RSQRT_MAGIC = 0x5F3759DF

# instance schedule: update(b) >= 3 slots after gate(b)
SEQ = [("g", 0), ("g", 1), ("g", 2), ("u", 0), ("g", 3), ("u", 1), ("g", 4),
       ("u", 2), ("g", 5), ("u", 3), ("g", 6), ("u", 4), ("g", 7), ("u", 5),
       ("u", 6), ("u", 7)]


def _pt(nt):
    return min(128, N - nt * 128)


def _build_body(tc, ctx, nc, P):
    def pool(name, bufs, space="SBUF"):
        return ctx.enter_context(tc.tile_pool(name=name, bufs=bufs, space=space))

    wp = pool("wp", 1)        # static weights
    dat = pool("dat", 2)      # per-batch DMA loads
    act = pool("act", 2)      # per-instance intermediates
    arp = pool("arp", 14)     # relu(A) tiles: 2 instances x 7 in flight
    xnp = pool("xnp", 29)     # natural xs/cand tiles
    xpp = pool("xpp", 15)     # d*xs tiles
    zcp = pool("zcp", 2)      # einsum contraction chunks
    dnp = pool("dnp", 4)      # rowsum/d helpers
    psp = pool("psp", 4, space="PSUM")  # op + yT accumulators (tag psA)
    psa = pool("psa", 4, space="PSUM")  # A halves / hypernet / transposes

    def ps_pair(p, name, parts, tag):
        return [p.tile([parts, sl], F32, tag=tag, name=f"{name}_{i}")
                for i, (s0, sl) in enumerate(SPLITS)]

    # ---------------- static setup phase A: small tiles the first
    # instance needs immediately (batch-0 input DMAs must not queue
    # behind the bulk weights) ------------------------------------
    ident_f = wp.tile([128, 128], F32, tag="identf", name="ident_f")
    nc.sync.dma_start(ident_f[:, :], P["ident"][:, :])
    ident_b = wp.tile([128, 128], BF16, tag="identb", name="ident_b")
    nc.vector.tensor_copy(ident_b[:, :], ident_f[:, :])
    ones66 = wp.tile([1, C], BF16, tag="ones66", name="ones66")
    nc.vector.memset(ones66[:, :], 1.0)

    def load_bf(pname, shape, tag):
        t = wp.tile(list(shape), BF16, tag=tag, name=pname + "_t")
        nc.sync.dma_start(t[:, :], P[pname][:, :])
        return t

    embT = load_bf("embT", (E, N), "embT")
    fc = {}
    for br in ("g", "u"):
        fc[("w1", br)] = load_bf(f"fc1w_{br}", (C, 16), f"fc1w{br}")
        fc[("w2", br)] = load_bf(f"fc2w_{br}", (16, 2), f"fc2w{br}")
        fc[("w3", br)] = load_bf(f"fc3w_{br}", (2, E), f"fc3w{br}")
        for nm, shape in (("b1", (16, 1)), ("b2", (2, 1)), ("b3", (E, 1))):
            t = wp.tile(list(shape), F32, tag=f"fc{nm}{br}", name=f"fc{nm}{br}")
            nc.sync.dma_start(t[:, :], P[f"fc{nm}_{br}"][:, :])
            fc[(nm, br)] = t

    def statics_phase_b():
        """Bulk weights: first needed by P6(SEQ[0]) ~2 slots in."""
        nonlocal embB128, embSS, wzc, wzs
        embB128 = []
        for e in range(E):
            t = wp.tile([128, N], BF16, tag=f"embB{e}", name=f"embB{e}")
            nc.sync.dma_start(t[:, :], P["embB128"][e * 128 : (e + 1) * 128, :])
            embB128.append(t)
        embSS = wp.tile([52, N], BF16, tag="embSS", name="embSS")
        nc.sync.dma_start(embSS[:, :], P["embSS"][:, :])
        wzc = {}
        wzs = {}
        for br, On in (("g", OG), ("u", OU)):
            tiles = []
            for e in range(E):
                t = wp.tile([128, On], BF16, tag=f"wzc{br}{e}", name=f"wzc{br}{e}")
                nc.sync.dma_start(t[:, :], P[f"wzc_{br}"][e * 128 : (e + 1) * 128, :])
                tiles.append(t)
            wzc[br] = tiles
            t = wp.tile([52, On], BF16, tag=f"wzs{br}", name=f"wzs{br}")
            nc.sync.dma_start(t[:, :], P[f"wzs_{br}"][:, :])
            wzs[br] = t

    embB128 = embSS = wzc = wzs = None

    # ---------------- per-instance state ----------------
    ST = {}   # (br,b) -> dict of tiles
    BAT = {}  # b -> dict of per-batch tiles

    def batch_load(b):
        """DMA this batch's inputs; build Mb, xgsx, and the gate XGS base."""
        d = {}
        xs_nat = []
        for nt in range(NT):
            p = _pt(nt)
            t = xnp.tile([128, C], BF16, tag="xsn", name=f"xsn{b}{nt}")
            nc.sync.dma_start(t[:p, :], P["xs_nat"][b, nt * 128 : nt * 128 + p, :])
            xs_nat.append(t)
        d["xs_nat"] = xs_nat
        xsTt = act.tile([C, N], BF16, tag="xsT", name=f"xsT{b}", bufs=4)
        nc.sync.dma_start(xsTt[:, :], P["xsT"][b, :, :])
        d["xsT"] = xsTt
        # gate-branch contraction stack: rows 0:64 = xs[2:66] straight from HBM
        xgs_g = act.tile([128, N], BF16, tag="xgsg", name=f"xgsg{b}", bufs=3)
        nc.sync.dma_start(xgs_g[0:64, :], P["xsT"][b, 2:C, :])
        d["xgs_g"] = xgs_g
        stT = dat.tile([DOUT, N], BF16, tag="stT", name=f"stT{b}", bufs=4)
        nc.sync.dma_start(stT[:, :], P["stateT"][b, :, :])
        d["stT"] = stT
        # x rows replicated per e for the small contraction chunk (both branches)
        xgsx = act.tile([20, N], BF16, tag="xgsx", name=f"xgsx{b}", bufs=4)
        nc.gpsimd.dma_start(xgsx[0:2, :], xsTt[0:DIN, :])
        nc.gpsimd.dma_start(xgsx[2:4, :], xgsx[0:2, :])
        nc.gpsimd.dma_start(xgsx[4:8, :], xgsx[0:4, :])
        nc.gpsimd.dma_start(xgsx[8:16, :], xgsx[0:8, :])
        nc.gpsimd.dma_start(xgsx[16:20, :], xgsx[0:4, :])
        d["xgsx"] = xgsx
        tdso = []
        for nm in ("tT", "dT", "sT", "oT"):
            t = dat.tile([E, N], BF16, tag=nm, name=f"{nm}{b}", bufs=2)
            nc.sync.dma_start(t[:, :], P[nm][b, :, :])
            tdso.append(t)
        p1 = act.tile([E, N], BF16, tag="p1", name=f"p1_{b}", bufs=2)
        nc.vector.tensor_mul(p1[:, :], tdso[0][:, :], tdso[1][:, :])
        p2 = act.tile([E, N], BF16, tag="p2", name=f"p2_{b}", bufs=1)
        nc.vector.tensor_mul(p2[:, :], tdso[2][:, :], tdso[3][:, :])
        p3 = act.tile([E, N], BF16, tag="p1", name=f"p3_{b}", bufs=2)
        nc.vector.tensor_mul(p3[:, :], p1[:, :], p2[:, :])
        Mb = act.tile([E, N], BF16, tag="Mb", name=f"Mb{b}", bufs=4)
        nc.vector.tensor_mul(Mb[:, :], p3[:, :], embT[:, :])
        d["Mb"] = Mb
        BAT[b] = d

    def P1(inst):
        """Hypernet + V. For gate instances, also triggers the batch load."""
        br, b = inst
        if br == "g":
            batch_load(b)
            st = ST[inst] = {}
            st["x0"] = BAT[b]["xsT"]
            st["xgs"] = BAT[b]["xgs_g"]
        else:
            st = ST[inst]  # created by glue(gate): has x0=candT, xgs, r_sb, cn
        xg2 = st["x0"]
        h1p = ps_pair(psa, f"h1p{br}{b}", 16, "psB")
        h1 = act.tile([16, N], BF16, tag="h1", name=f"h1{br}{b}")
        for i, (s0, sl) in enumerate(SPLITS):
            nc.tensor.matmul(h1p[i][:16, :sl], fc[("w1", br)][:, :],
                             xg2[:, s0 : s0 + sl], start=True, stop=True)
            nc.scalar.activation(h1[:, s0 : s0 + sl], h1p[i][:16, :sl],
                                 AF.Sigmoid, bias=fc[("b1", br)][:, :])
        h2p = ps_pair(psa, f"h2p{br}{b}", 2, "psB")
        h2 = act.tile([2, N], BF16, tag="h2", name=f"h2{br}{b}")
        for i, (s0, sl) in enumerate(SPLITS):
            nc.tensor.matmul(h2p[i][:2, :sl], fc[("w2", br)][:, :],
                             h1[:, s0 : s0 + sl], start=True, stop=True)
            nc.scalar.activation(h2[:, s0 : s0 + sl], h2p[i][:2, :sl],
                                 AF.Sigmoid, bias=fc[("b2", br)][:, :])
        h3p = ps_pair(psa, f"h3p{br}{b}", E, "psB")
        filt = act.tile([E, N], BF16, tag="filt", name=f"filt{br}{b}")
        for i, (s0, sl) in enumerate(SPLITS):
            nc.tensor.matmul(h3p[i][:E, :sl], fc[("w3", br)][:, :],
                             h2[:, s0 : s0 + sl], start=True, stop=True)
            nc.vector.tensor_scalar_add(filt[:, s0 : s0 + sl], h3p[i][:E, :sl],
                                        fc[("b3", br)][:, :])
        vpre = act.tile([E, N], BF16, tag="vpre", name=f"vpre{br}{b}")
        nc.vector.tensor_mul(vpre[:, :], BAT[b]["Mb"][:, :], filt[:, :])
        V = act.tile([E, N], BF16, tag="V", name=f"V{br}{b}")
        nc.scalar.activation(V[:, :], vpre[:, :], AF.Tanh)
        st["V"] = V
        rs0 = dnp.tile([128, 8], F32, tag="rs0", name=f"rs0{br}{b}")
        rs1 = dnp.tile([128, 8], F32, tag="rs1", name=f"rs1{br}{b}")
        nc.vector.memset(rs0[:, :], 0.5)
        nc.vector.memset(rs1[:, :], 0.5)
        st["rs"] = (rs0, rs1)

    def P2(inst):
        """A = relu(V V^T) + fused row-sums."""
        br, b = inst
        st = ST[inst]
        V, rsh = st["V"], st["rs"]
        ar = []
        for kt in range(NT):
            p = _pt(kt)
            aps = [psa.tile([128, sl], F32, tag="psB", name=f"aps{br}{b}{kt}_{i}")
                   for i, (s0, sl) in enumerate(SPLITS)]
            art = arp.tile([128, N], BF16, tag="ar", name=f"ar{br}{b}{kt}")
            for i, (s0, sl) in enumerate(SPLITS):
                nc.tensor.matmul(aps[i][:p, :sl],
                                 V[:, kt * 128 : kt * 128 + p],
                                 V[:, s0 : s0 + sl], start=True, stop=True)
                nc.scalar.activation(art[:p, s0 : s0 + sl], aps[i][:p, :sl],
                                     AF.Relu, accum_out=rsh[i][:p, kt : kt + 1])
            ar.append(art)
        st["ar"] = ar

    def P3(inst):
        """d = rsqrt(rowsums) on DVE; x' = d*xs; transposed d row (drs)."""
        br, b = inst
        st = ST[inst]
        rs0, rs1 = st["rs"]
        rsall = dnp.tile([128, 8], F32, tag="rsall", name=f"rsall{br}{b}")
        nc.vector.tensor_add(rsall[:, :], rs0[:, :], rs1[:, :])
        tsh = dnp.tile([128, 8], F32, tag="tsh", name=f"tsh{br}{b}")
        nc.vector.tensor_scalar(tsh[:, :].bitcast(I32), rsall[:, :].bitcast(I32),
                                1, None, AluOpType.logical_shift_right)
        tnot = dnp.tile([128, 8], F32, tag="tnot", name=f"tnot{br}{b}")
        nc.vector.tensor_scalar(tnot[:, :].bitcast(I32), tsh[:, :].bitcast(I32),
                                -1, None, AluOpType.bitwise_xor)
        d0 = dnp.tile([128, 8], F32, tag="d0", name=f"d0{br}{b}")
        nc.vector.tensor_scalar(d0[:, :].bitcast(I32), tnot[:, :].bitcast(I32),
                                RSQRT_MAGIC + 1, None, AluOpType.add)
        sq = dnp.tile([128, 8], F32, tag="sq", name=f"sq{br}{b}")
        nc.vector.tensor_mul(sq[:, :], d0[:, :], d0[:, :])
        hx = dnp.tile([128, 8], F32, tag="hx", name=f"hx{br}{b}")
        nc.vector.tensor_mul(hx[:, :], sq[:, :], rsall[:, :])
        cf = dnp.tile([128, 8], F32, tag="cf", name=f"cf{br}{b}")
        nc.vector.tensor_scalar(cf[:, :], hx[:, :], -0.5, 1.5,
                                AluOpType.mult, AluOpType.add)
        dcat = dnp.tile([128, 8], F32, tag="dcat", name=f"dcat{br}{b}")
        nc.vector.tensor_mul(dcat[:, :], d0[:, :], cf[:, :])
        st["dcat"] = dcat
        xnat = BAT[b]["xs_nat"] if br == "g" else st["cn"]
        xp = []
        for kt in range(NT):
            p = _pt(kt)
            xpt = xpp.tile([128, C], BF16, tag="xp", name=f"xp{br}{b}{kt}")
            nc.vector.tensor_scalar_mul(xpt[:p, :], xnat[kt][:p, :],
                                        dcat[:p, kt : kt + 1])
            xp.append(xpt)
        st["xp"] = xp
        # transposed d row, one slot ahead of P4's dB outer products
        tp = psa.tile([128, 128], F32, tag="psB", name=f"dtp{br}{b}")
        nc.tensor.transpose(tp[:8, :128], dcat[:, :], ident_f[:, :])
        drs = act.tile([8, 128], BF16, tag="drs", name=f"drs{br}{b}")
        nc.vector.tensor_copy(drs[:, :], tp[:8, :128])
        st["drs7"] = drs

    def P4(inst):
        """y^T matmuls + dB via per-row-tile outer products from drs."""
        br, b = inst
        st = ST[inst]
        drow = act.tile([1, 896], BF16, tag="drow", name=f"drow{br}{b}")
        nc.sync.dma_start(drow[0:1, :], st["drs7"][0:7, :])
        st["drow"] = drow
        yt = ps_pair(psp, f"yt{br}{b}", C, "psA")
        ar, xp = st["ar"], st["xp"]
        for kt in range(NT):
            p = _pt(kt)
            for i, (s0, sl) in enumerate(SPLITS):
                nc.tensor.matmul(yt[i][:C, :sl], xp[kt][:p, :],
                                 ar[kt][:p, s0 : s0 + sl],
                                 start=(kt == 0), stop=(kt == NT - 1))
        st["yt"] = yt
        drow = st["drow"]
        dbp = [psa.tile([128, sl], F32, tag="psB", name=f"dbp{br}{b}_{i}")
               for i, (s0, sl) in enumerate(SPLITS)]
        dB = act.tile([C, N], BF16, tag="dB", name=f"dB{br}{b}")
        for i, (s0, sl) in enumerate(SPLITS):
            nc.tensor.matmul(dbp[i][:C, :sl], ones66[:, :],
                             drow[0:1, s0 : s0 + sl], start=True, stop=True)
            nc.scalar.copy(dB[:, s0 : s0 + sl], dbp[i][:C, :sl])
        st["dB"] = dB

    def P5(inst):
        """Lx^T = xsT - dB*y^T; stack Lx rows into the contraction tiles."""
        br, b = inst
        st = ST[inst]
        yt, dB = st["yt"], st["dB"]
        yd = act.tile([C, N], BF16, tag="yd", name=f"yd{br}{b}")
        for i, (s0, sl) in enumerate(SPLITS):
            nc.vector.tensor_mul(yd[:, s0 : s0 + sl], yt[i][:C, :sl],
                                 dB[:, s0 : s0 + sl])
        lxT = act.tile([C, N], BF16, tag="lxT", name=f"lxT{br}{b}", bufs=3)
        nc.vector.tensor_sub(lxT[:, :], st["x0"][:, :], yd[:, :])
        st["x1"] = lxT
        nc.gpsimd.dma_start(st["xgs"][64:128, :], lxT[2:C, :])
        xg1 = act.tile([52, N], BF16, tag="xg1", name=f"xg1{br}{b}", bufs=3)
        nc.gpsimd.dma_start(xg1[32:34, :], lxT[0:DIN, :])
        nc.gpsimd.dma_start(xg1[34:36, :], xg1[32:34, :])
        nc.gpsimd.dma_start(xg1[36:40, :], xg1[32:36, :])
        nc.gpsimd.dma_start(xg1[40:48, :], xg1[32:40, :])
        nc.gpsimd.dma_start(xg1[48:52, :], xg1[32:36, :])
        st["xg1"] = xg1

    def P6(inst):
        """Final per-node einsum (128-row repacked contraction) + activation."""
        br, b = inst
        st = ST[inst]
        On = OG if br == "g" else OU
        outf = AF.Sigmoid if br == "g" else AF.Tanh
        xgs = st["xgs"]
        op = ps_pair(psp, f"op{br}{b}", On, "psA")
        for e in range(E):
            zc = zcp.tile([128, N], BF16, tag=f"zc{e % 2}", name=f"zc{br}{b}{e}")
            nc.vector.tensor_mul(zc[:, :], embB128[e][:, :], xgs[:, :])
            for i, (s0, sl) in enumerate(SPLITS):
                nc.tensor.matmul(op[i][:On, :sl], wzc[br][e][:, :On],
                                 zc[:, s0 : s0 + sl], start=(e == 0), stop=False)
        small = zcp.tile([52, N], BF16, tag="small", name=f"small{br}{b}")
        nc.vector.tensor_mul(small[0:20, :], embSS[0:20, :],
                             BAT[b]["xgsx"][0:20, :])
        nc.gpsimd.dma_start(small[20:32, :], P["embTz"][:, :])
        nc.vector.tensor_mul(small[32:52, :], embSS[32:52, :],
                             st["xg1"][32:52, :])
        for i, (s0, sl) in enumerate(SPLITS):
            nc.tensor.matmul(op[i][:On, :sl], wzs[br][0:52, :On],
                             small[0:52, s0 : s0 + sl], start=False, stop=True)
        zout = act.tile([On, N], BF16, tag=f"zout{br}", name=f"zout{br}{b}")
        for i, (s0, sl) in enumerate(SPLITS):
            nc.scalar.activation(zout[:, s0 : s0 + sl], op[i][:On, :sl], outf)
        st["zout"] = zout

    def glue(inst):
        """After P6: gate -> build update inputs; update -> epilogue + store."""
        br, b = inst
        if br == "g":
            zr = ST[inst]["zout"]  # (128, N): z rows 0:64, r rows 64:128
            stT = BAT[b]["stT"]
            ust = {}
            ST[("u", b)] = ust
            zst = act.tile([DOUT, N], BF16, tag="zst", name=f"zst{b}")
            nc.vector.tensor_mul(zst[:, :], zr[0:DOUT, :], stT[:, :])
            candT = act.tile([C, N], BF16, tag="candT", name=f"candT{b}", bufs=3)
            nc.gpsimd.dma_start(candT[0:DIN, :], BAT[b]["xsT"][0:DIN, :])
            nc.gpsimd.dma_start(candT[DIN:C, :], zst[:, :])
            ust["x0"] = candT
            xgs_u = act.tile([128, N], BF16, tag="xgsu", name=f"xgsu{b}", bufs=3)
            nc.gpsimd.dma_start(xgs_u[0:64, :], zst[:, :])
            ust["xgs"] = xgs_u
            r_sb = act.tile([DOUT, N], BF16, tag="r_sb", name=f"r_sb{b}", bufs=3)
            nc.gpsimd.dma_start(r_sb[:, :], zr[DOUT:OG, :])
            ust["r_sb"] = r_sb
            cn_l = []
            for nt in range(NT):
                p = _pt(nt)
                zps = psa.tile([128, 128], BF16, tag="psB", name=f"znp{b}{nt}")
                nc.tensor.transpose(zps[:p, :DOUT],
                                    zr[0:DOUT, nt * 128 : nt * 128 + p],
                                    ident_b[:DOUT, :DOUT])
                zn = act.tile([128, DOUT], BF16, tag="zn", name=f"zn{b}{nt}",
                              bufs=4)
                nc.vector.tensor_copy(zn[:p, :], zps[:p, :DOUT])
                cn = xnp.tile([128, C], BF16, tag="cn", name=f"cn{b}{nt}", bufs=22)
                nc.gpsimd.tensor_copy(cn[:p, 0:DIN],
                                      BAT[b]["xs_nat"][nt][:p, 0:DIN])
                nc.gpsimd.tensor_mul(cn[:p, DIN:C], zn[:p, :],
                                     BAT[b]["xs_nat"][nt][:p, DIN:C])
                cn_l.append(cn)
            ust["cn"] = cn_l
        else:
            hc = ST[inst]["zout"]  # (64, N)
            stT = BAT[b]["stT"]
            r_sb = ST[inst]["r_sb"]
            t1 = act.tile([OU, N], BF16, tag="t1", name=f"t1_{b}", bufs=2)
            nc.gpsimd.tensor_sub(t1[:, :], stT[:, :], hc[:, :])
            t2 = act.tile([OU, N], BF16, tag="t2", name=f"t2_{b}", bufs=2)
            nc.gpsimd.tensor_mul(t2[:, :], r_sb[:, :], t1[:, :])
            outT = act.tile([OU, N], F32, tag="outT", name=f"outT{b}")
            nc.vector.tensor_add(outT[:, :], t2[:, :], hc[:, :])
            nc.sync.dma_start(P["out"][b, :, :], outT[:, :])

    # ---------------- pipeline driver ----------------
    M = len(SEQ)
    P1(SEQ[0])
    for s in range(M + 2):
        if 0 <= s - 2 < M:
            P6(SEQ[s - 2])
            glue(SEQ[s - 2])
        if 0 <= s - 1 < M:
            P4(SEQ[s - 1])
            P5(SEQ[s - 1])
        if s < M:
            P2(SEQ[s])
            P3(SEQ[s])
        if s + 1 < M:
            P1(SEQ[s + 1])
        if s == 0:
            statics_phase_b()


def build_nc():
    nc = bacc.Bacc()
    P = {}

    def dp(name, shape, dtype=F32, out=False):
        P[name] = nc.declare_dram_parameter(name, list(shape), dtype, isOutput=out)

    dp("xs_nat", (BL, N, C), BF16)
    dp("xsT", (BL, C, N), BF16)
    dp("stateT", (BL, DOUT, N), BF16)
    for nm in ("tT", "dT", "sT", "oT"):
        dp(nm, (BL, E, N), BF16)
    dp("embT", (E, N), BF16)
    dp("embB128", (E * 128, N), BF16)
    dp("embSS", (52, N), BF16)
    dp("embTz", (12, N), BF16)
    dp("wzc_g", (E * 128, OG), BF16)
    dp("wzc_u", (E * 128, OU), BF16)
    dp("wzs_g", (52, OG), BF16)
    dp("wzs_u", (52, OU), BF16)
    for br in ("g", "u"):
        dp(f"fc1w_{br}", (C, 16), BF16)
        dp(f"fc2w_{br}", (16, 2), BF16)
        dp(f"fc3w_{br}", (2, E), BF16)
        dp(f"fcb1_{br}", (16, 1))
        dp(f"fcb2_{br}", (2, 1))
        dp(f"fcb3_{br}", (E, 1))
    dp("ident", (128, 128))
    dp("out", (BL, OU, N), F32, out=True)
    with tile.TileContext(nc) as tc:
        with ExitStack() as ctx:
            _build_body(tc, ctx, nc, P)
    nc.finalize()
    return nc


_NC_CACHE = {}


def _get_nc():
    if "nc" not in _NC_CACHE:
        _NC_CACHE["nc"] = build_nc()
    return _NC_CACHE["nc"]


def _make_in_maps(inputs):
    f32 = lambda a: np.ascontiguousarray(a, dtype=np.float32)
    bf = lambda a: np.ascontiguousarray(np.asarray(a, dtype=np.float32).astype(BF16_NP))
    x = f32(inputs["x"])
    state = f32(inputs["state"])
    emb = f32(inputs["node_embeddings"])
    time, day = f32(inputs["time"]), f32(inputs["day"])
    speed, occupy = f32(inputs["speed"]), f32(inputs["occupy"])
    xs = np.concatenate([x, state], axis=-1)
    embT = emb.T  # (E, N)
    embB128 = np.repeat(embT[:, None, :], 128, axis=1).reshape(E * 128, N)
    emb_pairs = np.repeat(embT[:, None, :], 2, axis=1).reshape(2 * E, N)
    embSS = np.concatenate([emb_pairs, np.zeros((12, N), np.float32), emb_pairs],
                           axis=0)
    embTz = np.concatenate([embT, np.zeros((2, N), np.float32)], axis=0)

    def pack_w(wpool, bpool, On):
        # main chunks: per e [wpool[e,0,2:66]; wpool[e,1,2:66]] -> (1280, On)
        wzc = np.concatenate(
            [np.concatenate([wpool[e, 0, DIN:C], wpool[e, 1, DIN:C]], axis=0)
             for e in range(E)], axis=0)
        # small chunk: x rows (k=0,c<2) per e, Lx rows (k=1,c<2) per e, bias
        w_x = wpool[:, 0, 0:DIN, :].reshape(E * DIN, On)
        w_l = wpool[:, 1, 0:DIN, :].reshape(E * DIN, On)
        wzs = np.concatenate([w_x, bpool, np.zeros((2, On), np.float32), w_l],
                             axis=0)
        return wzc, wzs

    wzc_g, wzs_g = pack_w(f32(inputs["gate_wpool"]), f32(inputs["gate_bpool"]), OG)
    wzc_u, wzs_u = pack_w(f32(inputs["update_wpool"]), f32(inputs["update_bpool"]), OU)


    shared = {
        "embT": bf(embT),
        "embB128": bf(embB128),
        "embSS": bf(embSS),
        "embTz": bf(embTz),
        "wzc_g": bf(wzc_g),
        "wzc_u": bf(wzc_u),
        "wzs_g": bf(wzs_g),
        "wzs_u": bf(wzs_u),
        "ident": np.eye(128, dtype=np.float32),
    }
    for br, pre in (("g", "gate"), ("u", "update")):
        shared[f"fc1w_{br}"] = bf(inputs[f"{pre}_fc1_w"])
        shared[f"fc2w_{br}"] = bf(inputs[f"{pre}_fc2_w"])
        shared[f"fc3w_{br}"] = bf(inputs[f"{pre}_fc3_w"])
        shared[f"fcb1_{br}"] = f32(inputs[f"{pre}_fc1_b"].reshape(16, 1))
        shared[f"fcb2_{br}"] = f32(inputs[f"{pre}_fc2_b"].reshape(2, 1))
        shared[f"fcb3_{br}"] = f32(inputs[f"{pre}_fc3_b"].reshape(E, 1))

    in_maps = []
    for c in range(NCORES):
        sl = slice(c * BL, (c + 1) * BL)
        m = dict(shared)
        m["xs_nat"] = bf(xs[sl])
        m["xsT"] = bf(xs[sl].transpose(0, 2, 1))
        m["stateT"] = bf(state[sl].transpose(0, 2, 1))
        m["tT"] = bf(time[sl].transpose(0, 2, 1))
        m["dT"] = bf(day[sl].transpose(0, 2, 1))
        m["sT"] = bf(speed[sl].transpose(0, 2, 1))
        m["oT"] = bf(occupy[sl].transpose(0, 2, 1))
        in_maps.append(m)
    return in_maps


def _run(inputs, trace=False):
    nc = _get_nc()
    in_maps = _make_in_maps(inputs)
    res = run_bass_kernel_spmd(nc, in_maps, core_ids=list(range(NCORES)), trace=trace)
    out = np.concatenate(
        [np.asarray(res.results[i]["out"]).transpose(0, 2, 1) for i in range(NCORES)],
        axis=0,
    )
    return out.astype(np.float32), res


def kernel(**inputs):
    out, _ = _run(inputs, trace=False)
    return out


# revision 15
# speedup vs baseline: 1.2184x; 1.0245x over previous
"""DDGCRN cell on 8 TRN2 NeuronCores — data-parallel over batch.

Per core: 8 batches = 16 branch-instances (gate O=128 / update O=64), emitted
as a software pipeline so every engine's static instruction stream stays
dense (engines execute their streams in order; serial per-instance chains
would otherwise stall the TensorEngine and re-throttle its HAM clock gate).

Pipeline: step s emits  P6(s-2) op-matmuls+activation | P4(s-1) dB+yT
| P5(s-1) Lx | P2(s) A-matmuls+relu+rowsum | P3(s) rsqrt+x'+dT | P1(s+1)
hypernet+V.  Update(b) is sequenced >=3 slots after gate(b) (needs z).

Math per instance:
  filt = hypernet MLP (transposed-feature layout, bf16)
  V = tanh(emb*time*day*speed*occupy*filt)      (10, 883)
  A = relu(V V^T) (883,883 symmetric) + fused row-sums (ACT accum_out)
  d = rsqrt(rowsum) via fast-inverse-sqrt + 1 Newton step (DVE only; keeps
      ScalarE pinned to the sigmoid/tanh/relu table set — no table reloads)
  Lx^T = xs^T - dB * ((d*xs)^T A)   using A's symmetry; dB built by 7
      per-row-tile outer products from the transposed d (no DMA broadcast)
  out^T: the einsum sum_{e,k,c} wpool[e,k,c,o] emb[n,e] xg_k[c,n] + bias is
      repacked into 128-row contraction chunks: per e one chunk
      [emb_e*xs[2:66]; emb_e*Lx[2:66]] (one DVE mul against a DMA-stacked
      [xs[2:66];Lx[2:66]] tile), plus one 50-row chunk holding the c<2 rows
      (x / Lx leading rows replicated per e via DMA) and the bias rows
      (emb^T direct).  11 accumulating matmuls per split instead of 21.

All matmuls bf16 (PSUM f32); inputs pre-cast/pre-transposed on host (pure
layout/dtype prep). Output written transposed, un-transposed on host.
"""

import sys, os

sys.path.insert(0, "/opt/trn_rl_repo")

import numpy as np
import ml_dtypes
from contextlib import ExitStack

import concourse.bass as bass
import concourse.bacc as bacc
import concourse.mybir as mybir
from concourse import tile
from concourse.alu_op_type import AluOpType
from concourse.bass_utils import run_bass_kernel_spmd

AF = mybir.ActivationFunctionType
F32 = mybir.dt.float32
BF16 = mybir.dt.bfloat16
FP8 = mybir.dt.float8e4
I32 = mybir.dt.int32
BF16_NP = ml_dtypes.bfloat16
FP8_NP = ml_dtypes.float8_e4m3
NPAD = 896

B, N, DIN, DOUT, E, CHEB = 64, 883, 2, 64, 10, 2
C = DIN + DOUT  # 66
NCORES = 8
BL = B // NCORES  # 8 batches per core
NT = (N + 127) // 128  # 7 row tiles
OG, OU = 2 * DOUT, DOUT  # 128, 64
SPLITS = [(0, 512), (512, N - 512)]
RSQRT_MAGIC = 0x5F3759DF

# instance schedule: update(b) >= 3 slots after gate(b)
SEQ = [("g", 0), ("g", 1), ("g", 2), ("u", 0), ("g", 3), ("u", 1), ("g", 4),
       ("u", 2), ("g", 5), ("u", 3), ("g", 6), ("u", 4), ("g", 7), ("u", 5),
       ("u", 6), ("u", 7)]


def _pt(nt):
    return min(128, N - nt * 128)


def _build_body(tc, ctx, nc, P):
    def pool(name, bufs, space="SBUF"):
        return ctx.enter_context(tc.tile_pool(name=name, bufs=bufs, space=space))

    wp = pool("wp", 1)        # static weights
    dat = pool("dat", 2)      # per-batch DMA loads
    act = pool("act", 2)      # per-instance intermediates
    arp = pool("arp", 14)     # relu(A) tiles: 2 instances x 7 in flight
    xnp = pool("xnp", 29)     # natural xs/cand tiles
    xpp = pool("xpp", 15)     # d*xs tiles
    zcp = pool("zcp", 2)      # einsum contraction chunks
    dnp = pool("dnp", 4)      # rowsum/d helpers
    psp = pool("psp", 4, space="PSUM")  # op + yT accumulators (tag psA)
    psa = pool("psa", 4, space="PSUM")  # A halves / hypernet / transposes

    def ps_pair(p, name, parts, tag):
        return [p.tile([parts, sl], F32, tag=tag, name=f"{name}_{i}")
                for i, (s0, sl) in enumerate(SPLITS)]

    # ---------------- static setup phase A: small tiles the first
    # instance needs immediately (batch-0 input DMAs must not queue
    # behind the bulk weights) ------------------------------------
    PRE = {}
    PRE["xsT"] = act.tile([C, N], BF16, tag="xsT", name="xsT0", bufs=4)
    nc.sync.dma_start(PRE["xsT"][:, :], P["xsT"][0, :, :])
    PRE["tdso"] = []
    for nm in ("tT", "dT", "sT", "oT"):
        t = dat.tile([E, N], BF16, tag=nm, name=f"{nm}0", bufs=2)
        nc.sync.dma_start(t[:, :], P[nm][0, :, :])
        PRE["tdso"].append(t)

    ident_f = wp.tile([128, 128], F32, tag="identf", name="ident_f")
    nc.sync.dma_start(ident_f[:, :], P["ident"][:, :])
    ident_b = wp.tile([128, 128], BF16, tag="identb", name="ident_b")
    nc.vector.tensor_copy(ident_b[:, :], ident_f[:, :])
    ones66 = wp.tile([1, C], BF16, tag="ones66", name="ones66")
    nc.vector.memset(ones66[:, :], 1.0)

    def load_bf(pname, shape, tag):
        t = wp.tile(list(shape), BF16, tag=tag, name=pname + "_t")
        nc.sync.dma_start(t[:, :], P[pname][:, :])
        return t

    embT = load_bf("embT", (E, N), "embT")
    fc = {}
    for br in ("g", "u"):
        fc[("w1", br)] = load_bf(f"fc1w_{br}", (C, 16), f"fc1w{br}")
        fc[("w2", br)] = load_bf(f"fc2w_{br}", (16, 2), f"fc2w{br}")
        fc[("w3", br)] = load_bf(f"fc3w_{br}", (2, E), f"fc3w{br}")
        for nm, shape in (("b1", (16, 1)), ("b2", (2, 1)), ("b3", (E, 1))):
            t = wp.tile(list(shape), F32, tag=f"fc{nm}{br}", name=f"fc{nm}{br}")
            nc.sync.dma_start(t[:, :], P[f"fc{nm}_{br}"][:, :])
            fc[(nm, br)] = t

    def statics_phase_b():
        """Bulk weights: first needed by P6(SEQ[0]) ~2 slots in."""
        nonlocal embB128, embSS, wzc, wzs
        embB128 = []
        for e in range(E):
            t = wp.tile([128, N], BF16, tag=f"embB{e}", name=f"embB{e}")
            nc.sync.dma_start(t[:, :], P["embB128"][e * 128 : (e + 1) * 128, :])
            embB128.append(t)
        embSS = wp.tile([52, N], BF16, tag="embSS", name="embSS")
        nc.sync.dma_start(embSS[:, :], P["embSS"][:, :])
        wzc = {}
        wzs = {}
        for br, On in (("g", OG), ("u", OU)):
            tiles = []
            for e in range(E):
                t = wp.tile([128, On], BF16, tag=f"wzc{br}{e}", name=f"wzc{br}{e}")
                nc.sync.dma_start(t[:, :], P[f"wzc_{br}"][e * 128 : (e + 1) * 128, :])
                tiles.append(t)
            wzc[br] = tiles
            t = wp.tile([52, On], BF16, tag=f"wzs{br}", name=f"wzs{br}")
            nc.sync.dma_start(t[:, :], P[f"wzs_{br}"][:, :])
            wzs[br] = t

    embB128 = embSS = wzc = wzs = None

    # ---------------- per-instance state ----------------
    ST = {}   # (br,b) -> dict of tiles
    BAT = {}  # b -> dict of per-batch tiles

    def batch_load(b):
        """DMA this batch's inputs; build Mb, xgsx, and the gate XGS base."""
        d = {}
        xs_nat = []
        for nt in range(NT):
            p = _pt(nt)
            t = xnp.tile([128, C], BF16, tag="xsn", name=f"xsn{b}{nt}")
            nc.sync.dma_start(t[:p, :], P["xs_nat"][b, nt * 128 : nt * 128 + p, :])
            xs_nat.append(t)
        d["xs_nat"] = xs_nat
        if b == 0:
            xsTt = PRE["xsT"]
        else:
            xsTt = act.tile([C, N], BF16, tag="xsT", name=f"xsT{b}", bufs=4)
            nc.sync.dma_start(xsTt[:, :], P["xsT"][b, :, :])
        d["xsT"] = xsTt
        # gate-branch contraction stack: rows 0:64 = xs[2:66] straight from HBM
        xgs_g = act.tile([128, N], BF16, tag="xgsg", name=f"xgsg{b}", bufs=3)
        nc.sync.dma_start(xgs_g[0:64, :], P["xsT"][b, 2:C, :])
        d["xgs_g"] = xgs_g
        stT = dat.tile([DOUT, N], BF16, tag="stT", name=f"stT{b}", bufs=4)
        nc.sync.dma_start(stT[:, :], P["stateT"][b, :, :])
        d["stT"] = stT
        # x rows replicated per e for the small contraction chunk (both branches)
        xgsx = act.tile([20, N], BF16, tag="xgsx", name=f"xgsx{b}", bufs=4)
        nc.gpsimd.dma_start(xgsx[0:2, :], xsTt[0:DIN, :])
        nc.gpsimd.dma_start(xgsx[2:4, :], xgsx[0:2, :])
        nc.gpsimd.dma_start(xgsx[4:8, :], xgsx[0:4, :])
        nc.gpsimd.dma_start(xgsx[8:16, :], xgsx[0:8, :])
        nc.gpsimd.dma_start(xgsx[16:20, :], xgsx[0:4, :])
        d["xgsx"] = xgsx
        if b == 0:
            tdso = PRE["tdso"]
        else:
            tdso = []
            for nm in ("tT", "dT", "sT", "oT"):
                t = dat.tile([E, N], BF16, tag=nm, name=f"{nm}{b}", bufs=2)
                nc.sync.dma_start(t[:, :], P[nm][b, :, :])
                tdso.append(t)
        p1 = act.tile([E, N], BF16, tag="p1", name=f"p1_{b}", bufs=2)
        nc.vector.tensor_mul(p1[:, :], tdso[0][:, :], tdso[1][:, :])
        p2 = act.tile([E, N], BF16, tag="p2", name=f"p2_{b}", bufs=1)
        nc.vector.tensor_mul(p2[:, :], tdso[2][:, :], tdso[3][:, :])
        p3 = act.tile([E, N], BF16, tag="p1", name=f"p3_{b}", bufs=2)
        nc.vector.tensor_mul(p3[:, :], p1[:, :], p2[:, :])
        Mb = act.tile([E, N], BF16, tag="Mb", name=f"Mb{b}", bufs=4)
        nc.vector.tensor_mul(Mb[:, :], p3[:, :], embT[:, :])
        d["Mb"] = Mb
        BAT[b] = d

    def P1(inst):
        """Hypernet + V. For gate instances, also triggers the batch load."""
        br, b = inst
        if br == "g":
            batch_load(b)
            st = ST[inst] = {}
            st["x0"] = BAT[b]["xsT"]
            st["xgs"] = BAT[b]["xgs_g"]
        else:
            st = ST[inst]  # created by glue(gate): has x0=candT, xgs, r_sb, cn
        xg2 = st["x0"]
        h1p = ps_pair(psa, f"h1p{br}{b}", 16, "psB")
        h1 = act.tile([16, N], BF16, tag="h1", name=f"h1{br}{b}")
        for i, (s0, sl) in enumerate(SPLITS):
            nc.tensor.matmul(h1p[i][:16, :sl], fc[("w1", br)][:, :],
                             xg2[:, s0 : s0 + sl], start=True, stop=True)
            nc.scalar.activation(h1[:, s0 : s0 + sl], h1p[i][:16, :sl],
                                 AF.Sigmoid, bias=fc[("b1", br)][:, :])
        h2p = ps_pair(psa, f"h2p{br}{b}", 2, "psB")
        h2 = act.tile([2, N], BF16, tag="h2", name=f"h2{br}{b}")
        for i, (s0, sl) in enumerate(SPLITS):
            nc.tensor.matmul(h2p[i][:2, :sl], fc[("w2", br)][:, :],
                             h1[:, s0 : s0 + sl], start=True, stop=True)
            nc.scalar.activation(h2[:, s0 : s0 + sl], h2p[i][:2, :sl],
                                 AF.Sigmoid, bias=fc[("b2", br)][:, :])
        h3p = ps_pair(psa, f"h3p{br}{b}", E, "psB")
        filt = act.tile([E, N], BF16, tag="filt", name=f"filt{br}{b}")
        for i, (s0, sl) in enumerate(SPLITS):
            nc.tensor.matmul(h3p[i][:E, :sl], fc[("w3", br)][:, :],
                             h2[:, s0 : s0 + sl], start=True, stop=True)
            nc.vector.tensor_scalar_add(filt[:, s0 : s0 + sl], h3p[i][:E, :sl],
                                        fc[("b3", br)][:, :])
        vpre = act.tile([E, N], BF16, tag="vpre", name=f"vpre{br}{b}")
        nc.vector.tensor_mul(vpre[:, :], BAT[b]["Mb"][:, :], filt[:, :])
        V = act.tile([E, N], BF16, tag="V", name=f"V{br}{b}")
        nc.scalar.activation(V[:, :], vpre[:, :], AF.Tanh)
        st["V"] = V
        rs0 = dnp.tile([128, 8], F32, tag="rs0", name=f"rs0{br}{b}")
        rs1 = dnp.tile([128, 8], F32, tag="rs1", name=f"rs1{br}{b}")
        nc.vector.memset(rs0[:, :], 0.5)
        nc.vector.memset(rs1[:, :], 0.5)
        st["rs"] = (rs0, rs1)

    def P2(inst):
        """A = relu(V V^T) + fused row-sums."""
        br, b = inst
        st = ST[inst]
        V, rsh = st["V"], st["rs"]
        ar = []
        for kt in range(NT):
            p = _pt(kt)
            aps = [psa.tile([128, sl], F32, tag="psB", name=f"aps{br}{b}{kt}_{i}")
                   for i, (s0, sl) in enumerate(SPLITS)]
            art = arp.tile([128, N], BF16, tag="ar", name=f"ar{br}{b}{kt}")
            for i, (s0, sl) in enumerate(SPLITS):
                nc.tensor.matmul(aps[i][:p, :sl],
                                 V[:, kt * 128 : kt * 128 + p],
                                 V[:, s0 : s0 + sl], start=True, stop=True)
                nc.scalar.activation(art[:p, s0 : s0 + sl], aps[i][:p, :sl],
                                     AF.Relu, accum_out=rsh[i][:p, kt : kt + 1])
            ar.append(art)
        st["ar"] = ar

    def P3(inst):
        """d = rsqrt(rowsums) on DVE; x' = d*xs; transposed d row (drs)."""
        br, b = inst
        st = ST[inst]
        rs0, rs1 = st["rs"]
        rsall = dnp.tile([128, 8], F32, tag="rsall", name=f"rsall{br}{b}")
        nc.vector.tensor_add(rsall[:, :], rs0[:, :], rs1[:, :])
        tsh = dnp.tile([128, 8], F32, tag="tsh", name=f"tsh{br}{b}")
        nc.vector.tensor_scalar(tsh[:, :].bitcast(I32), rsall[:, :].bitcast(I32),
                                1, None, AluOpType.logical_shift_right)
        tnot = dnp.tile([128, 8], F32, tag="tnot", name=f"tnot{br}{b}")
        nc.vector.tensor_scalar(tnot[:, :].bitcast(I32), tsh[:, :].bitcast(I32),
                                -1, None, AluOpType.bitwise_xor)
        d0 = dnp.tile([128, 8], F32, tag="d0", name=f"d0{br}{b}")
        nc.vector.tensor_scalar(d0[:, :].bitcast(I32), tnot[:, :].bitcast(I32),
                                RSQRT_MAGIC + 1, None, AluOpType.add)
        sq = dnp.tile([128, 8], F32, tag="sq", name=f"sq{br}{b}")
        nc.vector.tensor_mul(sq[:, :], d0[:, :], d0[:, :])
        hx = dnp.tile([128, 8], F32, tag="hx", name=f"hx{br}{b}")
        nc.vector.tensor_mul(hx[:, :], sq[:, :], rsall[:, :])
        cf = dnp.tile([128, 8], F32, tag="cf", name=f"cf{br}{b}")
        nc.vector.tensor_scalar(cf[:, :], hx[:, :], -0.5, 1.5,
                                AluOpType.mult, AluOpType.add)
        dcat = dnp.tile([128, 8], F32, tag="dcat", name=f"dcat{br}{b}")
        nc.vector.tensor_mul(dcat[:, :], d0[:, :], cf[:, :])
        st["dcat"] = dcat
        xnat = BAT[b]["xs_nat"] if br == "g" else st["cn"]
        xp = []
        for kt in range(NT):
            p = _pt(kt)
            xpt = xpp.tile([128, C], BF16, tag="xp", name=f"xp{br}{b}{kt}")
            nc.vector.tensor_scalar_mul(xpt[:p, :], xnat[kt][:p, :],
                                        dcat[:p, kt : kt + 1])
            xp.append(xpt)
        st["xp"] = xp
        # transposed d row, one slot ahead of P4's dB outer products
        tp = psa.tile([128, 128], F32, tag="psB", name=f"dtp{br}{b}")
        nc.tensor.transpose(tp[:8, :128], dcat[:, :], ident_f[:, :])
        drs = act.tile([8, 128], BF16, tag="drs", name=f"drs{br}{b}")
        nc.vector.tensor_copy(drs[:, :], tp[:8, :128])
        st["drs7"] = drs

    def P4(inst):
        """y^T matmuls + dB via per-row-tile outer products from drs."""
        br, b = inst
        st = ST[inst]
        drow = act.tile([1, 896], BF16, tag="drow", name=f"drow{br}{b}")
        nc.gpsimd.dma_start(drow[0:1, :], st["drs7"][0:7, :])
        st["drow"] = drow
        yt = ps_pair(psp, f"yt{br}{b}", C, "psA")
        ar, xp = st["ar"], st["xp"]
        for kt in range(NT):
            p = _pt(kt)
            for i, (s0, sl) in enumerate(SPLITS):
                nc.tensor.matmul(yt[i][:C, :sl], xp[kt][:p, :],
                                 ar[kt][:p, s0 : s0 + sl],
                                 start=(kt == 0), stop=(kt == NT - 1))
        st["yt"] = yt
        drow = st["drow"]
        dbp = [psa.tile([128, sl], F32, tag="psB", name=f"dbp{br}{b}_{i}")
               for i, (s0, sl) in enumerate(SPLITS)]
        dB = act.tile([C, N], BF16, tag="dB", name=f"dB{br}{b}")
        for i, (s0, sl) in enumerate(SPLITS):
            nc.tensor.matmul(dbp[i][:C, :sl], ones66[:, :],
                             drow[0:1, s0 : s0 + sl], start=True, stop=True)
            nc.scalar.copy(dB[:, s0 : s0 + sl], dbp[i][:C, :sl])
        st["dB"] = dB

    def P5(inst):
        """Lx^T = xsT - dB*y^T; stack Lx rows into the contraction tiles."""
        br, b = inst
        st = ST[inst]
        yt, dB = st["yt"], st["dB"]
        yd = act.tile([C, N], BF16, tag="yd", name=f"yd{br}{b}")
        for i, (s0, sl) in enumerate(SPLITS):
            nc.vector.tensor_mul(yd[:, s0 : s0 + sl], yt[i][:C, :sl],
                                 dB[:, s0 : s0 + sl])
        lxT = act.tile([C, N], BF16, tag="lxT", name=f"lxT{br}{b}", bufs=3)
        nc.vector.tensor_sub(lxT[:, :], st["x0"][:, :], yd[:, :])
        st["x1"] = lxT
        nc.gpsimd.dma_start(st["xgs"][64:128, :], lxT[2:C, :])
        xg1 = act.tile([52, N], BF16, tag="xg1", name=f"xg1{br}{b}", bufs=3)
        nc.gpsimd.dma_start(xg1[32:34, :], lxT[0:DIN, :])
        nc.gpsimd.dma_start(xg1[34:36, :], xg1[32:34, :])
        nc.gpsimd.dma_start(xg1[36:40, :], xg1[32:36, :])
        nc.gpsimd.dma_start(xg1[40:48, :], xg1[32:40, :])
        nc.gpsimd.dma_start(xg1[48:52, :], xg1[32:36, :])
        st["xg1"] = xg1

    def P6(inst):
        """Final per-node einsum (128-row repacked contraction) + activation."""
        br, b = inst
        st = ST[inst]
        On = OG if br == "g" else OU
        outf = AF.Sigmoid if br == "g" else AF.Tanh
        xgs = st["xgs"]
        op = ps_pair(psp, f"op{br}{b}", On, "psA")
        for e in range(E):
            zc = zcp.tile([128, N], BF16, tag=f"zc{e % 2}", name=f"zc{br}{b}{e}")
            nc.vector.tensor_mul(zc[:, :], embB128[e][:, :], xgs[:, :])
            for i, (s0, sl) in enumerate(SPLITS):
                nc.tensor.matmul(op[i][:On, :sl], wzc[br][e][:, :On],
                                 zc[:, s0 : s0 + sl], start=(e == 0), stop=False)
        small = zcp.tile([52, N], BF16, tag="small", name=f"small{br}{b}")
        nc.vector.tensor_mul(small[0:20, :], embSS[0:20, :],
                             BAT[b]["xgsx"][0:20, :])
        nc.gpsimd.dma_start(small[20:32, :], P["embTz"][:, :])
        nc.vector.tensor_mul(small[32:52, :], embSS[32:52, :],
                             st["xg1"][32:52, :])
        for i, (s0, sl) in enumerate(SPLITS):
            nc.tensor.matmul(op[i][:On, :sl], wzs[br][0:52, :On],
                             small[0:52, s0 : s0 + sl], start=False, stop=True)
        zout = act.tile([On, N], BF16, tag=f"zout{br}", name=f"zout{br}{b}")
        for i, (s0, sl) in enumerate(SPLITS):
            nc.scalar.activation(zout[:, s0 : s0 + sl], op[i][:On, :sl], outf)
        st["zout"] = zout

    def glue(inst):
        """After P6: gate -> build update inputs; update -> epilogue + store."""
        br, b = inst
        if br == "g":
            zr = ST[inst]["zout"]  # (128, N): z rows 0:64, r rows 64:128
            stT = BAT[b]["stT"]
            ust = {}
            ST[("u", b)] = ust
            zst = act.tile([DOUT, N], BF16, tag="zst", name=f"zst{b}")
            nc.vector.tensor_mul(zst[:, :], zr[0:DOUT, :], stT[:, :])
            candT = act.tile([C, N], BF16, tag="candT", name=f"candT{b}", bufs=3)
            nc.gpsimd.dma_start(candT[0:DIN, :], BAT[b]["xsT"][0:DIN, :])
            nc.gpsimd.dma_start(candT[DIN:C, :], zst[:, :])
            ust["x0"] = candT
            xgs_u = act.tile([128, N], BF16, tag="xgsu", name=f"xgsu{b}", bufs=3)
            nc.gpsimd.dma_start(xgs_u[0:64, :], zst[:, :])
            ust["xgs"] = xgs_u
            r_sb = act.tile([DOUT, N], BF16, tag="r_sb", name=f"r_sb{b}", bufs=3)
            nc.gpsimd.dma_start(r_sb[:, :], zr[DOUT:OG, :])
            ust["r_sb"] = r_sb
            cn_l = []
            for nt in range(NT):
                p = _pt(nt)
                zps = psa.tile([128, 128], BF16, tag="psB", name=f"znp{b}{nt}")
                nc.tensor.transpose(zps[:p, :DOUT],
                                    zr[0:DOUT, nt * 128 : nt * 128 + p],
                                    ident_b[:DOUT, :DOUT])
                zn = act.tile([128, DOUT], BF16, tag="zn", name=f"zn{b}{nt}",
                              bufs=4)
                nc.vector.tensor_copy(zn[:p, :], zps[:p, :DOUT])
                cn = xnp.tile([128, C], BF16, tag="cn", name=f"cn{b}{nt}", bufs=22)
                nc.gpsimd.tensor_copy(cn[:p, 0:DIN],
                                      BAT[b]["xs_nat"][nt][:p, 0:DIN])
                nc.gpsimd.tensor_mul(cn[:p, DIN:C], zn[:p, :],
                                     BAT[b]["xs_nat"][nt][:p, DIN:C])
                cn_l.append(cn)
            ust["cn"] = cn_l
        else:
            hc = ST[inst]["zout"]  # (64, N)
            stT = BAT[b]["stT"]
            r_sb = ST[inst]["r_sb"]
            t1 = act.tile([OU, N], BF16, tag="t1", name=f"t1_{b}", bufs=2)
            nc.gpsimd.tensor_sub(t1[:, :], stT[:, :], hc[:, :])
            t2 = act.tile([OU, N], BF16, tag="t2", name=f"t2_{b}", bufs=2)
            nc.gpsimd.tensor_mul(t2[:, :], r_sb[:, :], t1[:, :])
            outT = act.tile([OU, N], F32, tag="outT", name=f"outT{b}")
            nc.vector.tensor_add(outT[:, :], t2[:, :], hc[:, :])
            nc.sync.dma_start(P["out"][b, :, :], outT[:, :])

    # ---------------- pipeline driver ----------------
    M = len(SEQ)
    P1(SEQ[0])
    for s in range(M + 2):
        if 0 <= s - 2 < M:
            P6(SEQ[s - 2])
            glue(SEQ[s - 2])
        if 0 <= s - 1 < M:
            P4(SEQ[s - 1])
            P5(SEQ[s - 1])
        if s < M:
            P2(SEQ[s])
            P3(SEQ[s])
        if s + 1 < M:
            P1(SEQ[s + 1])
        if s == 0:
            statics_phase_b()


def build_nc():
    nc = bacc.Bacc()
    P = {}

    def dp(name, shape, dtype=F32, out=False):
        P[name] = nc.declare_dram_parameter(name, list(shape), dtype, isOutput=out)

    dp("xs_nat", (BL, N, C), BF16)
    dp("xsT", (BL, C, N), BF16)
    dp("stateT", (BL, DOUT, N), BF16)
    for nm in ("tT", "dT", "sT", "oT"):
        dp(nm, (BL, E, N), BF16)
    dp("embT", (E, N), BF16)
    dp("embB128", (E * 128, N), BF16)
    dp("embSS", (52, N), BF16)
    dp("embTz", (12, N), BF16)
    dp("wzc_g", (E * 128, OG), BF16)
    dp("wzc_u", (E * 128, OU), BF16)
    dp("wzs_g", (52, OG), BF16)
    dp("wzs_u", (52, OU), BF16)
    for br in ("g", "u"):
        dp(f"fc1w_{br}", (C, 16), BF16)
        dp(f"fc2w_{br}", (16, 2), BF16)
        dp(f"fc3w_{br}", (2, E), BF16)
        dp(f"fcb1_{br}", (16, 1))
        dp(f"fcb2_{br}", (2, 1))
        dp(f"fcb3_{br}", (E, 1))
    dp("ident", (128, 128))
    dp("out", (BL, OU, N), F32, out=True)
    with tile.TileContext(nc) as tc:
        with ExitStack() as ctx:
            _build_body(tc, ctx, nc, P)
    nc.finalize()
    return nc


_NC_CACHE = {}


def _get_nc():
    if "nc" not in _NC_CACHE:
        _NC_CACHE["nc"] = build_nc()
    return _NC_CACHE["nc"]


def _make_in_maps(inputs):
    f32 = lambda a: np.ascontiguousarray(a, dtype=np.float32)
    bf = lambda a: np.ascontiguousarray(np.asarray(a, dtype=np.float32).astype(BF16_NP))
    x = f32(inputs["x"])
    state = f32(inputs["state"])
    emb = f32(inputs["node_embeddings"])
    time, day = f32(inputs["time"]), f32(inputs["day"])
    speed, occupy = f32(inputs["speed"]), f32(inputs["occupy"])
    xs = np.concatenate([x, state], axis=-1)
    embT = emb.T  # (E, N)
    embB128 = np.repeat(embT[:, None, :], 128, axis=1).reshape(E * 128, N)
    emb_pairs = np.repeat(embT[:, None, :], 2, axis=1).reshape(2 * E, N)
    embSS = np.concatenate([emb_pairs, np.zeros((12, N), np.float32), emb_pairs],
                           axis=0)
    embTz = np.concatenate([embT, np.zeros((2, N), np.float32)], axis=0)

    def pack_w(wpool, bpool, On):
        # main chunks: per e [wpool[e,0,2:66]; wpool[e,1,2:66]] -> (1280, On)
        wzc = np.concatenate(
            [np.concatenate([wpool[e, 0, DIN:C], wpool[e, 1, DIN:C]], axis=0)
             for e in range(E)], axis=0)
        # small chunk: x rows (k=0,c<2) per e, Lx rows (k=1,c<2) per e, bias
        w_x = wpool[:, 0, 0:DIN, :].reshape(E * DIN, On)
        w_l = wpool[:, 1, 0:DIN, :].reshape(E * DIN, On)
        wzs = np.concatenate([w_x, bpool, np.zeros((2, On), np.float32), w_l],
                             axis=0)
        return wzc, wzs

    wzc_g, wzs_g = pack_w(f32(inputs["gate_wpool"]), f32(inputs["gate_bpool"]), OG)
    wzc_u, wzs_u = pack_w(f32(inputs["update_wpool"]), f32(inputs["update_bpool"]), OU)


    shared = {
        "embT": bf(embT),
        "embB128": bf(embB128),
        "embSS": bf(embSS),
        "embTz": bf(embTz),
        "wzc_g": bf(wzc_g),
        "wzc_u": bf(wzc_u),
        "wzs_g": bf(wzs_g),
        "wzs_u": bf(wzs_u),
        "ident": np.eye(128, dtype=np.float32),
    }
    for br, pre in (("g", "gate"), ("u", "update")):
        shared[f"fc1w_{br}"] = bf(inputs[f"{pre}_fc1_w"])
        shared[f"fc2w_{br}"] = bf(inputs[f"{pre}_fc2_w"])
        shared[f"fc3w_{br}"] = bf(inputs[f"{pre}_fc3_w"])
        shared[f"fcb1_{br}"] = f32(inputs[f"{pre}_fc1_b"].reshape(16, 1))
        shared[f"fcb2_{br}"] = f32(inputs[f"{pre}_fc2_b"].reshape(2, 1))
        shared[f"fcb3_{br}"] = f32(inputs[f"{pre}_fc3_b"].reshape(E, 1))

    in_maps = []
    for c in range(NCORES):
        sl = slice(c * BL, (c + 1) * BL)
        m = dict(shared)
        m["xs_nat"] = bf(xs[sl])
        m["xsT"] = bf(xs[sl].transpose(0, 2, 1))
        m["stateT"] = bf(state[sl].transpose(0, 2, 1))
        m["tT"] = bf(time[sl].transpose(0, 2, 1))
        m["dT"] = bf(day[sl].transpose(0, 2, 1))
        m["sT"] = bf(speed[sl].transpose(0, 2, 1))
        m["oT"] = bf(occupy[sl].transpose(0, 2, 1))
        in_maps.append(m)
    return in_maps


def _run(inputs, trace=False):
    nc = _get_nc()
    in_maps = _make_in_maps(inputs)
    res = run_bass_kernel_spmd(nc, in_maps, core_ids=list(range(NCORES)), trace=trace)
    out = np.concatenate(
        [np.asarray(res.results[i]["out"]).transpose(0, 2, 1) for i in range(NCORES)],
        axis=0,
    )
    return out.astype(np.float32), res


def kernel(**inputs):
    out, _ = _run(inputs, trace=False)
    return out
